# revision 1
# baseline (speedup 1.0000x reference)
"""Multi-head attention (unstabilized softmax) on 8 TRN2 NeuronCores.

Reference computes, per (batch, head):
    scores  = Q @ K^T / sqrt(d)          [S, S]
    weights = exp(scores) / rowsum(exp)  (unstabilized softmax)
    out     = weights @ V                [S, d]

Sharding: B*H = 64 (batch, head) pairs split across 8 cores -> 8 heads per
core, fully independent (no collectives).

Device algorithm per head (S=2048, d=128):
  mm1: scoresT[k, q] = sum_d K[k,d] Q[q,d] with d on partitions.
       lhsT = K^T chunk [d=128, 128k] (stationary), rhs = Q^T [d=128, q]
       (host supplies Q^T/K^T layout [d, S]; bf16 matmul by default,
       ATTN_MM1_F32R=1 switches to fp32r for extra mm1 precision).
  exp: ACT reads scoresT tiles from PSUM, computes exp(scale*x), writes
       bf16 W^T tiles to SBUF (softmax numerator, transposed layout).
  mm2: out[q, 0:128] = sum_k W^T[k,q] * V[k,d];  col 128 = rowsum via a
       ones-column appended to V.  lhsT = W^T chunk (stationary, bf16+FWL),
       rhs = [V | 1] chunk [128k, 129].
  epilogue: DVE reciprocal of rowsum col + per-partition scalar multiply,
       DMA out f32.
"""

import math
import os

import numpy as np

import concourse.bass as bass  # noqa: F401  (bass types used via APs)
import concourse.mybir as mybir
from concourse import bacc
from concourse.tile import TileContext
from concourse.bass_utils import run_bass_kernel_spmd

B, H, S, D = 4, 16, 2048, 128
N_CORES = 8
HPC = (B * H) // N_CORES  # heads per core
SCALE = 1.0 / math.sqrt(D)

LAST_EXEC_TIME_NS = None
LAST_RESULTS = None
_NC_CACHE = {}


def build(hpc=HPC, s=S, mm1_f32r=True, exp_width=1024):
    """Build the per-core Bass graph. All cores run the same graph."""
    f32 = mybir.dt.float32
    f32r = mybir.dt.float32r
    bf16 = mybir.dt.bfloat16

    ktn = s // 128   # number of 128-row k tiles
    ew = min(exp_width, s)  # exp/activation tile width (PSUM-resident scores)
    assert ew % 512 == 0 and s % ew == 0
    ewn = s // ew

    nc = bacc.Bacc(None, target_bir_lowering=False)

    qk_dt = f32r if mm1_f32r else f32
    qt_d = nc.declare_dram_parameter("qt", [hpc, D, s], qk_dt, isOutput=False)
    kt_d = nc.declare_dram_parameter("kt", [hpc, D, s], qk_dt, isOutput=False)
    v_d = nc.declare_dram_parameter("v", [hpc, s, D], f32, isOutput=False)
    o_d = nc.declare_dram_parameter("out", [hpc, s, D], f32, isOutput=True)

    qh_n = hpc * ewn  # total q-half periods (pipeline granularity)

    with TileContext(nc) as tc:
        with (
            tc.tile_pool(name="qkstage", bufs=1) as qkstage_pool,
            tc.tile_pool(name="qk", bufs=2) as qk_pool,
            tc.tile_pool(name="vio", bufs=2) as vio_pool,
            tc.tile_pool(name="vaug", bufs=2) as vaug_pool,
            tc.tile_pool(name="wt", bufs=3) as wt_pool,
            tc.tile_pool(name="osb", bufs=4) as osb_pool,
            tc.tile_pool(name="rc", bufs=4) as rc_pool,
            tc.tile_pool(name="scoreps", bufs=3, space="PSUM") as score_pool,
            tc.tile_pool(name="outps", bufs=2, space="PSUM") as out_ps_pool,
        ):
            head_state = {}

            def load_head(h):
                """DMA + cast head h inputs; returns (q_mm, k_mm, v_aug)."""
                if mm1_f32r:
                    q_mm = qk_pool.tile([128, s], qk_dt, tag="q")
                    k_mm = qk_pool.tile([128, s], qk_dt, tag="k")
                    nc.sync.dma_start(out=q_mm, in_=qt_d[h])
                    nc.sync.dma_start(out=k_mm, in_=kt_d[h])
                else:
                    q_sb = qkstage_pool.tile([128, s], f32, tag="qs")
                    k_sb = qkstage_pool.tile([128, s], f32, tag="ks")
                    q_mm = qk_pool.tile([128, s], bf16, tag="q")
                    k_mm = qk_pool.tile([128, s], bf16, tag="k")
                    # Chunked DMA+cast so the first mm1 (low k/q columns) can
                    # start before the full tensors land. Head 0 gates the
                    # whole pipeline, so chunk it finest (startup latency).
                    nch = 4 if h == 0 else 2
                    cs = s // nch
                    for ci in range(nch):
                        c0 = ci * cs
                        nc.sync.dma_start(
                            out=k_sb[:, c0 : c0 + cs], in_=kt_d[h, :, c0 : c0 + cs]
                        )
                        nc.vector.tensor_copy(
                            out=k_mm[:, c0 : c0 + cs], in_=k_sb[:, c0 : c0 + cs]
                        )
                        nc.sync.dma_start(
                            out=q_sb[:, c0 : c0 + cs], in_=qt_d[h, :, c0 : c0 + cs]
                        )
                        nc.vector.tensor_copy(
                            out=q_mm[:, c0 : c0 + cs], in_=q_sb[:, c0 : c0 + cs]
                        )

                v_sb = vio_pool.tile([128, ktn, D], f32, tag="v")
                nc.sync.dma_start(
                    out=v_sb, in_=v_d[h].rearrange("(kt p) d -> p kt d", p=128)
                )
                v_aug = vaug_pool.tile([128, ktn, D + 1], bf16, tag="vaug")
                nc.vector.memset(v_aug[:, :, D : D + 1], 1.0)
                nc.vector.tensor_copy(out=v_aug[:, :, 0:D], in_=v_sb)
                return q_mm, k_mm, v_aug

            def mm1_exp(per, filler=()):
                """mm1 + exp for q-half period `per`; returns wt half tile.

                `filler` is a list of closures (pending mm2 q-tile emitters)
                interleaved one-per-kt so the PE stream has mm2 work while it
                is PSUM-throttled behind ACT.
                """
                h, ei = divmod(per, ewn)
                if per == 0:
                    head_state[0] = load_head(0)
                # Prefetch the next head's load+cast one period before it is
                # consumed: gives the DMA a full period of slack so the casts
                # never head-of-line-block the DVE epilogue ops behind them.
                nxt_per = per + 1
                if nxt_per < qh_n and nxt_per % ewn == 0:
                    nh = nxt_per // ewn
                    if nh not in head_state:
                        head_state[nh] = load_head(nh)
                q_mm, k_mm, _ = head_state[h]
                wt = wt_pool.tile([128, ktn, ew], bf16, tag="wt")
                fill_iter = iter(filler)
                for kt in range(ktn):
                    ps = score_pool.tile([128, ew], f32, tag="score")
                    for sub in range(ew // 512):
                        q0 = ei * ew + sub * 512
                        nc.tensor.matmul(
                            out=ps[:, sub * 512 : (sub + 1) * 512],
                            lhsT=k_mm[:, kt * 128 : (kt + 1) * 128],
                            rhs=q_mm[:, q0 : q0 + 512],
                            start=True,
                            stop=True,
                        )
                    nc.scalar.activation(
                        out=wt[:, kt, :],
                        in_=ps,
                        func=mybir.ActivationFunctionType.Exp,
                        scale=SCALE,
                    )
                    nxt = next(fill_iter, None)
                    if nxt is not None:
                        nxt()
                for nxt in fill_iter:
                    nxt()
                return wt

            def emit_mm2_qtile(per, wt, qi):
                """mm2 + normalize + store for one 128-row q tile."""
                h, ei = divmod(per, ewn)
                v_aug = head_state[h][2]
                qg = ei * (ew // 128) + qi  # global q-tile in head
                po = out_ps_pool.tile([128, D + 1], f32, tag="po")
                for kt in range(ktn):
                    nc.tensor.matmul(
                        out=po,
                        lhsT=wt[:, kt, qi * 128 : (qi + 1) * 128],
                        rhs=v_aug[:, kt, :],
                        start=(kt == 0),
                        stop=(kt == ktn - 1),
                    )
                rc = rc_pool.tile([128, 1], f32, tag="rc")
                nc.vector.reciprocal(out=rc, in_=po[:, D : D + 1])
                o_sb = osb_pool.tile([128, D], f32, tag="osb")
                nc.vector.tensor_scalar_mul(o_sb, po[:, 0:D], rc)
                # Last head's stores go on the (by-then idle) HWDGE sync queue
                # instead of SWDGE: SWDGE first-byte latency sits on the tail.
                store_eng = nc.sync if h == hpc - 1 else nc.gpsimd
                store_eng.dma_start(out=o_d[h, qg * 128 : (qg + 1) * 128, :], in_=o_sb)

            def mm2_epilogue(per, wt):
                for qi in range(ew // 128):
                    emit_mm2_qtile(per, wt, qi)

            # Software pipeline over q-half periods, distance 2: emit period
            # i's mm1/exp before period (i-2)'s mm2 so the in-order PE stream
            # always has matmul work while ACT catches up on exp.
            state = {}
            for per in range(qh_n):
                state[per] = mm1_exp(per)
                if per >= 2:
                    mm2_epilogue(per - 2, state.pop(per - 2))
            for p in sorted(state):
                mm2_epilogue(p, state.pop(p))

    return nc


def _shard_host(Q, K, V, hpc, n_cores):
    """Host-side shard + layout: returns per-core input maps."""
    BH = Q.shape[0] * Q.shape[1]
    s, d = Q.shape[2], Q.shape[3]
    Qf = np.ascontiguousarray(Q.reshape(BH, s, d))
    Kf = np.ascontiguousarray(K.reshape(BH, s, d))
    Vf = np.ascontiguousarray(V.reshape(BH, s, d))
    in_maps = []
    for c in range(n_cores):
        sl = slice(c * hpc, (c + 1) * hpc)
        in_maps.append(
            {
                "qt": np.ascontiguousarray(Qf[sl].transpose(0, 2, 1)),
                "kt": np.ascontiguousarray(Kf[sl].transpose(0, 2, 1)),
                "v": Vf[sl],
            }
        )
    return in_maps


def kernel(Q, K, V):
    global LAST_EXEC_TIME_NS, LAST_RESULTS
    Q = np.asarray(Q, dtype=np.float32)
    K = np.asarray(K, dtype=np.float32)
    V = np.asarray(V, dtype=np.float32)

    mm1_f32r = os.environ.get("ATTN_MM1_F32R", "0") == "1"
    trace = os.environ.get("ATTN_TRACE", "0") == "1"

    key = (HPC, S, mm1_f32r)
    nc = _NC_CACHE.get(key)
    if nc is None:
        nc = build(hpc=HPC, s=S, mm1_f32r=mm1_f32r)
        nc.compile()
        _NC_CACHE[key] = nc

    in_maps = _shard_host(Q, K, V, HPC, N_CORES)
    res = run_bass_kernel_spmd(nc, in_maps, core_ids=list(range(N_CORES)), trace=trace)
    LAST_EXEC_TIME_NS = res.exec_time_ns
    LAST_RESULTS = res

    out = np.concatenate([res.results[c]["out"] for c in range(N_CORES)], axis=0)
    return np.ascontiguousarray(out.reshape(B, H, S, D))



# revision 3
# speedup vs baseline: 1.0359x; 1.0359x over previous
"""Multi-head attention (unstabilized softmax) on 8 TRN2 NeuronCores.

Reference computes, per (batch, head):
    scores  = Q @ K^T / sqrt(d)          [S, S]
    weights = exp(scores) / rowsum(exp)  (unstabilized softmax)
    out     = weights @ V                [S, d]

Sharding: B*H = 64 (batch, head) pairs split across 8 cores -> 8 heads per
core, fully independent (no collectives).

Device pipeline per head (S=2048, d=128), q-chunk-major:
  For each q-chunk qc (512 q columns), for each k-tile kt (128 rows):
    mm1 block: scoresT[kt, qc] = K_chunk^T.T @ Q^T[:, qc]  -> PSUM [128, 512]
  Blocks land in 2 ping-ponged PSUM slots of [128, 3, 512] (3 banks each);
  ACT exp consumes 3 blocks per instruction (N=1536; plus one N=512
  remainder per phase) -> bf16 W^T chunks in SBUF.  Larger ACT tiles
  amortize the ~180-cycle per-instruction overhead (ACT is the critical
  engine: exp throughput is 1 elem/lane/cycle @ 1.2 GHz).
  mm2 (interleaved as PE filler): per 128-row q-tile, accumulate over kt:
    po[q,0:129] += W^T[kt,q].T @ [V|1][kt]   (ones col -> rowsum)
  epilogue: DVE reciprocal of col 128, per-partition scalar multiply,
  DMA out f32.  mm2 for q-chunk qc runs during the exp phase of qc+1, so
  the pipeline tail is only the final q-chunk's mm2 (~4us vs ~19us for
  head-major ordering).

Host prep: Q^T/K^T layouts [d, S] in bf16, V augmented with a ones column
([S, 129] bf16) so no device-side casts are needed.
"""

import math
import os

import numpy as np
import ml_dtypes

import concourse.bass as bass  # noqa: F401  (bass types used via APs)
import concourse.mybir as mybir
from concourse import bacc
from concourse.tile import TileContext
from concourse.bass_utils import run_bass_kernel_spmd

B, H, S, D = 4, 16, 2048, 128
N_CORES = 8
HPC = (B * H) // N_CORES  # heads per core
SCALE = 1.0 / math.sqrt(D)

KT = S // 128          # 16 k-tiles per head
QC = S // 512          # 4 q-chunks per head
BLK = 512              # score block = one mm1 matmul (N=512, one PSUM bank)
STITCH = 3             # blocks per exp instruction (3 banks per PSUM slot)

LAST_EXEC_TIME_NS = None
LAST_RESULTS = None
_NC_CACHE = {}


def build(hpc=HPC):
    f32 = mybir.dt.float32
    bf16 = mybir.dt.bfloat16

    nc = bacc.Bacc(None, target_bir_lowering=False)

    qt_d = nc.declare_dram_parameter("qt", [hpc, D, S], bf16, isOutput=False)
    kt_d = nc.declare_dram_parameter("kt", [hpc, D, S], bf16, isOutput=False)
    va_d = nc.declare_dram_parameter("va", [hpc, S, D + 1], bf16, isOutput=False)
    o_d = nc.declare_dram_parameter("out", [hpc, S, D], f32, isOutput=True)

    # exp stitch plan per phase: (kt_start, n_blocks)
    plan = []
    kt0 = 0
    while kt0 < KT:
        nb = min(STITCH, KT - kt0)
        plan.append((kt0, nb))
        kt0 += nb

    with TileContext(nc) as tc:
        with (
            tc.tile_pool(name="qk", bufs=2) as qk_pool,
            tc.tile_pool(name="va", bufs=2) as va_pool,
            tc.tile_pool(name="wt", bufs=3) as wt_pool,
            tc.tile_pool(name="osb", bufs=4) as osb_pool,
            tc.tile_pool(name="rc", bufs=4) as rc_pool,
            tc.tile_pool(name="scoreps", bufs=2, space="PSUM") as score_pool,
            tc.tile_pool(name="outps", bufs=2, space="PSUM") as out_ps_pool,
        ):
            head_state = {}

            def load_head(h):
                """DMA head h inputs (bf16, no casts needed)."""
                q_sb = qk_pool.tile([128, S], bf16, tag="q")
                k_sb = qk_pool.tile([128, S], bf16, tag="k")
                if h == 0:
                    # Chunk finest: the first exp needs only K cols 0:384
                    # and Q cols 0:512; don't gate on the full tensors.
                    for ci in range(4):
                        c0 = ci * 512
                        nc.sync.dma_start(
                            out=k_sb[:, c0 : c0 + 512], in_=kt_d[h, :, c0 : c0 + 512]
                        )
                        nc.sync.dma_start(
                            out=q_sb[:, c0 : c0 + 512], in_=qt_d[h, :, c0 : c0 + 512]
                        )
                else:
                    nc.sync.dma_start(out=k_sb, in_=kt_d[h])
                    nc.sync.dma_start(out=q_sb, in_=qt_d[h])
                va_sb = va_pool.tile([128, KT, D + 1], bf16, tag="va")
                nc.sync.dma_start(
                    out=va_sb, in_=va_d[h].rearrange("(kt p) d -> p kt d", p=128)
                )
                return q_sb, k_sb, va_sb

            def mm2_closures(h, qc, wt):
                """Flat list of closures: 64 mm2 matmuls + 4 epilogues."""
                _, _, va_sb = head_state[h]
                out = []
                for qi in range(4):
                    qg = qc * 4 + qi
                    po_box = {}

                    def mk_mm(kt, qi=qi, po_box=po_box):
                        def go():
                            if kt == 0:
                                po = out_ps_pool.tile([128, D + 1], f32, tag="po")
                                po_box["po"] = po
                            nc.tensor.matmul(
                                out=po_box["po"],
                                lhsT=wt[:, kt, qi * 128 : (qi + 1) * 128],
                                rhs=va_sb[:, kt, :],
                                start=(kt == 0),
                                stop=(kt == KT - 1),
                            )
                        return go

                    def mk_epi(qg=qg, po_box=po_box):
                        def go():
                            po = po_box["po"]
                            rc = rc_pool.tile([128, 1], f32, tag="rc")
                            nc.vector.reciprocal(out=rc, in_=po[:, D : D + 1])
                            o_sb = osb_pool.tile([128, D], f32, tag="osb")
                            nc.vector.tensor_scalar_mul(o_sb, po[:, 0:D], rc)
                            # Last head's stores on the idle HWDGE sync queue.
                            store_eng = nc.sync if h == hpc - 1 else nc.gpsimd
                            store_eng.dma_start(
                                out=o_d[h, qg * 128 : (qg + 1) * 128, :], in_=o_sb
                            )
                        return go

                    for kt in range(KT):
                        out.append(mk_mm(kt))
                    out.append(mk_epi())
                return out

            def emit_phase(h, qc, fillers):
                """mm1 + exp for (h, qc); interleave filler closures."""
                if (h, qc) == (0, 0):
                    head_state[0] = load_head(0)
                if qc == QC - 1 and h + 1 < hpc:
                    head_state[h + 1] = load_head(h + 1)
                q_sb, k_sb, _ = head_state[h]
                wt = wt_pool.tile([128, KT, 512], bf16, tag="wt")
                q0 = qc * 512

                fill_iter = iter(fillers)
                n_units = len(plan)
                for ui, (kt0, nb) in enumerate(plan):
                    ps = score_pool.tile([128, STITCH, BLK], f32, tag="score")
                    for j in range(nb):
                        kt = kt0 + j
                        nc.tensor.matmul(
                            out=ps[:, j, :],
                            lhsT=k_sb[:, kt * 128 : (kt + 1) * 128],
                            rhs=q_sb[:, q0 : q0 + 512],
                            start=True,
                            stop=True,
                        )
                    nc.scalar.activation(
                        out=wt[:, kt0 : kt0 + nb, :],
                        in_=ps[:, 0:nb, :],
                        func=mybir.ActivationFunctionType.Exp,
                        scale=SCALE,
                    )
                    # Interleave mm2 of the previous phase so the PE stream
                    # has matmul work while ACT drains the exp.
                    quota = 12 if nb == STITCH else 8
                    for _ in range(quota):
                        nxt = next(fill_iter, None)
                        if nxt is None:
                            break
                        nxt()
                for nxt in fill_iter:
                    nxt()
                return wt

            prev = None  # (h, qc, wt) awaiting mm2
            for h in range(hpc):
                for qc in range(QC):
                    fillers = mm2_closures(*prev) if prev is not None else []
                    wt = emit_phase(h, qc, fillers)
                    prev = (h, qc, wt)
            for cl in mm2_closures(*prev):
                cl()

    return nc


def _shard_host(Q, K, V, hpc, n_cores):
    """Host-side shard + layout + cast: returns per-core input maps."""
    bf16 = ml_dtypes.bfloat16
    BH = Q.shape[0] * Q.shape[1]
    s, d = Q.shape[2], Q.shape[3]
    Qf = Q.reshape(BH, s, d)
    Kf = K.reshape(BH, s, d)
    Vf = V.reshape(BH, s, d)
    Va = np.empty((BH, s, d + 1), dtype=bf16)
    Va[:, :, 0:d] = Vf.astype(bf16)
    Va[:, :, d] = 1.0
    in_maps = []
    for c in range(n_cores):
        sl = slice(c * hpc, (c + 1) * hpc)
        in_maps.append(
            {
                "qt": np.ascontiguousarray(
                    Qf[sl].transpose(0, 2, 1).astype(bf16)
                ),
                "kt": np.ascontiguousarray(
                    Kf[sl].transpose(0, 2, 1).astype(bf16)
                ),
                "va": Va[sl],
            }
        )
    return in_maps


def kernel(Q, K, V):
    global LAST_EXEC_TIME_NS, LAST_RESULTS
    Q = np.asarray(Q, dtype=np.float32)
    K = np.asarray(K, dtype=np.float32)
    V = np.asarray(V, dtype=np.float32)

    trace = os.environ.get("ATTN_TRACE", "0") == "1"

    key = (HPC, S)
    nc = _NC_CACHE.get(key)
    if nc is None:
        nc = build(hpc=HPC)
        nc.compile()
        _NC_CACHE[key] = nc

    in_maps = _shard_host(Q, K, V, HPC, N_CORES)
    res = run_bass_kernel_spmd(nc, in_maps, core_ids=list(range(N_CORES)), trace=trace)
    LAST_EXEC_TIME_NS = res.exec_time_ns
    LAST_RESULTS = res

    out = np.concatenate([res.results[c]["out"] for c in range(N_CORES)], axis=0)
    return np.ascontiguousarray(out.reshape(B, H, S, D))


# revision 10
# speedup vs baseline: 1.0585x; 1.0218x over previous
"""Multi-head attention (unstabilized softmax) on 8 TRN2 NeuronCores.

Reference computes, per (batch, head):
    scores  = Q @ K^T / sqrt(d)          [S, S]
    weights = exp(scores) / rowsum(exp)  (unstabilized softmax)
    out     = weights @ V                [S, d]

Sharding: B*H = 64 (batch, head) pairs split across 8 cores -> 8 heads per
core, fully independent (no collectives).

Device pipeline per head (S=2048, d=128), q-chunk-major:
  For each q-chunk qc (512 q columns), for each k-tile kt (128 rows):
    mm1 block: scoresT[kt, qc] = K_chunk^T.T @ Q^T[:, qc]  -> PSUM [128, 512]
  Blocks land in 2 ping-ponged PSUM slots of [128, 3, 512] (3 banks each);
  ACT exp consumes 3 blocks per instruction (N=1536; plus one N=512
  remainder per phase) -> bf16 W^T chunks in SBUF.  Larger ACT tiles
  amortize the ~180-cycle per-instruction overhead (ACT is the critical
  engine: exp throughput is 1 elem/lane/cycle @ 1.2 GHz).
  mm2 (interleaved as PE filler): per 128-row q-tile, accumulate over kt:
    po[q,0:129] += W^T[kt,q].T @ [V|1][kt]   (ones col -> rowsum)
  epilogue: DVE reciprocal of col 128, per-partition scalar multiply,
  DMA out f32.  mm2 for q-chunk qc runs during the exp phase of qc+1, so
  the pipeline tail is only the final q-chunk's mm2 (~4us vs ~19us for
  head-major ordering).

Host prep: Q^T/K^T layouts [d, S] in bf16, V augmented with a ones column
([S, 129] bf16) so no device-side casts are needed.
"""

import math
import os

import numpy as np
import ml_dtypes

import concourse.bass as bass  # noqa: F401  (bass types used via APs)
import concourse.mybir as mybir
from concourse import bacc
from concourse.tile import TileContext
from concourse.bass_utils import run_bass_kernel_spmd

B, H, S, D = 4, 16, 2048, 128
N_CORES = 8
HPC = (B * H) // N_CORES  # heads per core
SCALE = 1.0 / math.sqrt(D)

KT = S // 128          # 16 k-tiles per head
QC = S // 512          # 4 q-chunks per head
BLK = 512              # score block = one mm1 matmul (N=512, one PSUM bank)
STITCH = 3             # blocks per exp instruction (3 banks per PSUM slot)

LAST_EXEC_TIME_NS = None
LAST_RESULTS = None
_NC_CACHE = {}


def build(hpc=HPC):
    f32 = mybir.dt.float32
    bf16 = mybir.dt.bfloat16

    nc = bacc.Bacc(None, target_bir_lowering=False)

    # va/out use partition-major DRAM layouts so each partition's DMA run is
    # contiguous (big descriptors; <64KB strided transfers are
    # descriptor-dominated on the SDMA engines).
    qt_d = nc.declare_dram_parameter("qt", [hpc, D, S], bf16, isOutput=False)
    kt_d = nc.declare_dram_parameter("kt", [hpc, D, S], bf16, isOutput=False)
    va_d = nc.declare_dram_parameter("va", [hpc, 128, KT, D + 1], bf16, isOutput=False)
    o_d = nc.declare_dram_parameter("out", [hpc, 128, KT, D], f32, isOutput=True)

    # exp stitch plan per phase: (kt_start, n_blocks).  The trailing units
    # are 2-blocks each (not 3+1): every exp must be longer than the next
    # unit's mm1 refill (3 blocks = ~650ns) or ACT bubbles at the phase seam.
    plan = [(0, 3), (3, 3), (6, 3), (9, 3), (12, 2), (14, 2)]
    quotas = [13, 13, 13, 13, 9, 7]

    with TileContext(nc) as tc:
        with (
            tc.tile_pool(name="qk", bufs=2) as qk_pool,
            tc.tile_pool(name="va", bufs=2) as va_pool,
            tc.tile_pool(name="wt", bufs=3) as wt_pool,
            tc.tile_pool(name="osb", bufs=4) as osb_pool,
            tc.tile_pool(name="rc", bufs=4) as rc_pool,
            tc.tile_pool(name="scoreps", bufs=2, space="PSUM") as score_pool,
            tc.tile_pool(name="outps", bufs=2, space="PSUM") as out_ps_pool,
        ):
            head_state = {}

            def load_head(h):
                """DMA head h inputs (bf16, no casts needed)."""
                q_sb = qk_pool.tile([128, S], bf16, tag="q")
                k_sb = qk_pool.tile([128, S], bf16, tag="k")
                if h == 0:
                    # Chunk finest: the first exp needs only K cols 0:384
                    # and Q cols 0:512; don't gate on the full tensors.
                    for ci in range(4):
                        c0 = ci * 512
                        nc.sync.dma_start(
                            out=k_sb[:, c0 : c0 + 512], in_=kt_d[h, :, c0 : c0 + 512]
                        )
                        nc.sync.dma_start(
                            out=q_sb[:, c0 : c0 + 512], in_=qt_d[h, :, c0 : c0 + 512]
                        )
                else:
                    nc.sync.dma_start(out=k_sb, in_=kt_d[h])
                    nc.sync.dma_start(out=q_sb, in_=qt_d[h])
                va_sb = va_pool.tile([128, KT, D + 1], bf16, tag="va")
                nc.sync.dma_start(out=va_sb, in_=va_d[h])
                return q_sb, k_sb, va_sb

            def mm2_closures(h, qc, wt):
                """Flat list of closures: 64 mm2 matmuls + 4 epilogues.

                Output for the whole phase (4 q-tiles) collects in one SBUF
                tile and ships as a single 256KB store (big descriptors,
                fewer completion waits on the tail).
                """
                _, _, va_sb = head_state[h]
                shared = {}
                out = []
                for qi in range(4):
                    po_box = {}

                    def mk_mm(kt, qi=qi, po_box=po_box):
                        def go():
                            if kt == 0:
                                po = out_ps_pool.tile([128, D + 1], f32, tag="po")
                                po_box["po"] = po
                            nc.tensor.matmul(
                                out=po_box["po"],
                                lhsT=wt[:, kt, qi * 128 : (qi + 1) * 128],
                                rhs=va_sb[:, kt, :],
                                start=(kt == 0),
                                stop=(kt == KT - 1),
                            )
                        return go

                    def mk_epi(qi=qi, po_box=po_box):
                        def go():
                            po = po_box["po"]
                            if qi == 0:
                                o_sb = osb_pool.tile([128, 4, D], f32, tag="osb")
                                shared["o_sb"] = o_sb
                            rc = rc_pool.tile([128, 1], f32, tag="rc")
                            nc.vector.reciprocal(out=rc, in_=po[:, D : D + 1])
                            nc.vector.tensor_scalar_mul(
                                shared["o_sb"][:, qi, :], po[:, 0:D], rc
                            )
                            if qi == 3:
                                # Last head's stores on the idle HWDGE queue.
                                store_eng = nc.sync if h == hpc - 1 else nc.gpsimd
                                store_eng.dma_start(
                                    out=o_d[h, :, qc * 4 : (qc + 1) * 4, :],
                                    in_=shared["o_sb"],
                                )
                        return go

                    for kt in range(KT):
                        out.append(mk_mm(kt))
                    out.append(mk_epi())
                return out

            def emit_phase(h, qc, fillers):
                """mm1 + exp for (h, qc); interleave filler closures."""
                if (h, qc) == (0, 0):
                    head_state[0] = load_head(0)
                if qc == QC - 1 and h + 1 < hpc:
                    head_state[h + 1] = load_head(h + 1)
                q_sb, k_sb, _ = head_state[h]
                wt = wt_pool.tile([128, KT, 512], bf16, tag="wt")
                q0 = qc * 512

                fill_iter = iter(fillers)
                for ui, (kt0, nb) in enumerate(plan):
                    ps = score_pool.tile([128, STITCH, BLK], f32, tag="score")
                    for j in range(nb):
                        kt = kt0 + j
                        nc.tensor.matmul(
                            out=ps[:, j, :],
                            lhsT=k_sb[:, kt * 128 : (kt + 1) * 128],
                            rhs=q_sb[:, q0 : q0 + 512],
                            start=True,
                            stop=True,
                        )
                    nc.scalar.activation(
                        out=wt[:, kt0 : kt0 + nb, :],
                        in_=ps[:, 0:nb, :],
                        func=mybir.ActivationFunctionType.Exp,
                        scale=SCALE,
                    )
                    # Interleave mm2 of the previous phase so the PE stream
                    # has matmul work while ACT drains the exp.
                    for _ in range(quotas[ui]):
                        nxt = next(fill_iter, None)
                        if nxt is None:
                            break
                        nxt()
                for nxt in fill_iter:
                    nxt()
                return wt

            prev = None  # (h, qc, wt) awaiting mm2
            for h in range(hpc):
                for qc in range(QC):
                    fillers = mm2_closures(*prev) if prev is not None else []
                    wt = emit_phase(h, qc, fillers)
                    prev = (h, qc, wt)
            for cl in mm2_closures(*prev):
                cl()

    return nc


def _shard_host(Q, K, V, hpc, n_cores):
    """Host-side shard + layout + cast: returns per-core input maps."""
    bf16 = ml_dtypes.bfloat16
    BH = Q.shape[0] * Q.shape[1]
    s, d = Q.shape[2], Q.shape[3]
    kt_n = s // 128
    Qf = Q.reshape(BH, s, d)
    Kf = K.reshape(BH, s, d)
    Vf = V.reshape(BH, s, d)
    # Partition-major [h, p, kt, d+1]: per-partition DMA runs are contiguous.
    Va = np.empty((BH, 128, kt_n, d + 1), dtype=bf16)
    Va[:, :, :, 0:d] = Vf.reshape(BH, kt_n, 128, d).transpose(0, 2, 1, 3).astype(bf16)
    Va[:, :, :, d] = 1.0
    in_maps = []
    for c in range(n_cores):
        sl = slice(c * hpc, (c + 1) * hpc)
        in_maps.append(
            {
                "qt": np.ascontiguousarray(
                    Qf[sl].transpose(0, 2, 1).astype(bf16)
                ),
                "kt": np.ascontiguousarray(
                    Kf[sl].transpose(0, 2, 1).astype(bf16)
                ),
                "va": Va[sl],
            }
        )
    return in_maps


def kernel(Q, K, V):
    global LAST_EXEC_TIME_NS, LAST_RESULTS
    Q = np.asarray(Q, dtype=np.float32)
    K = np.asarray(K, dtype=np.float32)
    V = np.asarray(V, dtype=np.float32)

    trace = os.environ.get("ATTN_TRACE", "0") == "1"

    key = (HPC, S)
    nc = _NC_CACHE.get(key)
    if nc is None:
        nc = build(hpc=HPC)
        nc.compile()
        _NC_CACHE[key] = nc

    in_maps = _shard_host(Q, K, V, HPC, N_CORES)
    res = run_bass_kernel_spmd(nc, in_maps, core_ids=list(range(N_CORES)), trace=trace)
    LAST_EXEC_TIME_NS = res.exec_time_ns
    LAST_RESULTS = res

    # Device out layout is partition-major [hpc, p, qt, d] -> [hpc, S, D].
    out = np.concatenate([res.results[c]["out"] for c in range(N_CORES)], axis=0)
    out = out.reshape(B * H, 128, KT, D).transpose(0, 2, 1, 3)
    return np.ascontiguousarray(out.reshape(B, H, S, D))
